# revision 42
# baseline (speedup 1.0000x reference)
"""Trainium2 Bass kernel for retrieval_knn (nn_CLI_63702954934481).

Data-parallel over batch B=8: one batch per NeuronCore. Per core:
7-row bf16 PE matmul computes key = -d2 exactly in PSUM f32 (a^2 and b^2
split into bf16-exact byte halves). Selection: pairwise tensor_tensor max
tree (round 1 f32 PSUM -> bf16 SBUF, later rounds bf16 at 2x DVE rate),
hw top-8 max/max_index over group maxima, exact re-rank of the top NSEL
groups' candidates with full (d2, index) tie-breaking, indirect-DMA
gathers, weighted sum, concat. Self-contained.
"""
import sys

if "/opt/trn_rl_repo" not in sys.path:
    sys.path.insert(0, "/opt/trn_rl_repo")

import numpy as np
from concourse import bacc
from concourse.bass_utils import run_bass_kernel_spmd

from contextlib import ExitStack

import concourse.bass as bass
import concourse.mybir as mybir
import concourse.tile as tile
from concourse._compat import with_exitstack
from concourse.alu_op_type import AluOpType

P = 128
NA = 4096
NB = 4096
D = 64
K = 3
NT = NA // P      # 32 a-tiles
G = 16            # group size along b
NG = NB // G      # 256 groups
import os
NSEL = 4          # groups re-ranked per a-row
NC = NSEL * G     # 64 candidates
FS = 128.0
R = 0.5

MM_N = 512        # matmul chunk (one PSUM bank)
HALF = 2048       # b-columns per PSUM tile
HG = HALF // G    # 128 groups per half
import os
ND = int(os.environ.get("KND", "32"))  # groups per half drained by DVE reduce
IDX_DOUBLE = os.environ.get("KIDXD", "2")
POOL_B1 = os.environ.get("KPOOLB1", "0") == "1"
TREE_HALF = os.environ.get("KTREEH", "0") == "1"
CA = HG - ND      # groups per half converted to bf16 by Act (Pool can't
CT = CA           # read PSUM; the drain is DVE+Act only)

TB = int(os.environ.get("KTB", "8"))  # tiles per gather chunk
NCH = NT // TB    # chunks
SQ = TB * NSEL    # qb-window slots per chunk
SF = TB * K       # feature slots per chunk

f32 = mybir.dt.float32
i32 = mybir.dt.int32
i16 = mybir.dt.int16
u32 = mybir.dt.uint32
bf16 = mybir.dt.bfloat16
Act = mybir.ActivationFunctionType
X = mybir.AxisListType.X


@with_exitstack
def knn_kernel(ctx: ExitStack, tc: tile.TileContext, outs, ins):
    nc = tc.nc
    a_feats, b_feats, a_coords, b_coords = ins
    out = outs[0]

    prep_pool = ctx.enter_context(tc.tile_pool(name="prep", bufs=1))
    row_pool = ctx.enter_context(tc.tile_pool(name="rows", bufs=1))
    psum_pool = ctx.enter_context(tc.tile_pool(name="ps", bufs=2, space="PSUM"))
    sp = ctx.enter_context(tc.tile_pool(name="small", bufs=int(os.environ.get("KSPB", "4"))))

    # ---------------- Phase 0: prep ----------------
    # lhsT rows (per a): [2ax, 2ay, 2az, ah, al, 1, 1]
    # rhs rows (per b):  [bx, by, bz, -256, -1, -256*bh, -bl]
    # PSUM key = 2a.b - a^2 - b^2 = -d2 (exact f32 integer)
    qb_dram = nc.dram_tensor("qb_scratch", [NG, G * 4], f32)

    def load_q(coords, tag):
        c = prep_pool.tile([P, NT * 3], i32, name=f"c_{tag}")
        nc.sync.dma_start(c[:], coords.rearrange("(t p) d -> p t d", p=P))
        q_i = prep_pool.tile([P, NT * 3], i32, name=f"qi_{tag}")
        nc.vector.tensor_scalar(q_i[:], c[:], 4, None, op0=AluOpType.arith_shift_right)
        q_f = prep_pool.tile([P, NT * 3], f32, name=f"qf_{tag}")
        nc.vector.tensor_copy(q_f[:], q_i[:])
        return q_f

    def sq_sum(q_f, tag):
        sq = prep_pool.tile([P, NT * 3], f32, name=f"sq_{tag}")
        nc.vector.tensor_tensor(out=sq[:], in0=q_f[:], in1=q_f[:], op=AluOpType.mult)
        s2 = prep_pool.tile([P, NT], f32, name=f"s2_{tag}")
        nc.vector.tensor_reduce(out=s2[:], in_=sq[:].rearrange("p (t d) -> p t d", d=3),
                                axis=X, op=AluOpType.add)
        return s2

    def split_bytes(s2, tag):
        # s2 integer-valued f32 in [0, 49152): hi = s2>>8, lo = s2&255 (f32)
        s2_i = prep_pool.tile([P, NT], i32, name=f"s2i_{tag}")
        nc.vector.tensor_copy(s2_i[:], s2[:])
        hi_i = prep_pool.tile([P, NT], i32, name=f"hi_{tag}")
        nc.vector.tensor_scalar(hi_i[:], s2_i[:], 8, None, op0=AluOpType.arith_shift_right)
        lo_i = prep_pool.tile([P, NT], i32, name=f"lo_{tag}")
        nc.vector.tensor_scalar(lo_i[:], s2_i[:], 255, None, op0=AluOpType.bitwise_and)
        hi_f = prep_pool.tile([P, NT], f32, name=f"hif_{tag}")
        nc.vector.tensor_copy(hi_f[:], hi_i[:])
        lo_f = prep_pool.tile([P, NT], f32, name=f"lof_{tag}")
        nc.vector.tensor_copy(lo_f[:], lo_i[:])
        return hi_f, lo_f

    def stage_rows(vals_scales, const_rows, tag):
        # build [P, NT, 32] bf16 staging: col r holds row-r value per point;
        # stride 32 so transposed slices land on legal partition bases
        stg = prep_pool.tile([P, NT, 32], bf16, name=f"stg_{tag}")
        sv = stg[:]
        nc.gpsimd.memset(stg[:].rearrange("p t r -> p (t r)"), 0.0)
        for r, src_f, kk, dsel, scale in vals_scales:
            if kk == 1:
                view = src_f[:]
            else:
                view = src_f[:].rearrange("p (t d) -> p t d", d=kk)[:, :, dsel]
            nc.vector.tensor_scalar(sv[:, :, r], view, scale, None,
                                    op0=AluOpType.mult)
        for r, val in const_rows:
            nc.vector.memset(sv[:, :, r], val)
        # 8 XBAR transposes -> [(t r), p], 4 point-tiles per transpose
        trs = []
        for i in range(NT * 32 // P):
            tr = row_pool.tile([P, P], bf16, name=f"tr{i}_{tag}")
            nc.sync.dma_start_transpose(
                tr[:], stg[:].rearrange("p t r -> p (t r)")[:, i * P:(i + 1) * P])
            trs.append(tr)
        return trs

    nc.sync.dma_start(out[:, 0:D], a_feats[:])
    qa_f = load_q(a_coords, "a")
    qb_f = load_q(b_coords, "b")
    a2 = sq_sum(qa_f, "a")
    b2 = sq_sum(qb_f, "b")
    ah, al = split_bytes(a2, "a")
    bh, bl = split_bytes(b2, "b")

    # lhsT rows: [2ax, 2ay, 2az, ah, al, 1, 1]
    ta = stage_rows(
        [(0, qa_f, 3, 0, 2.0), (1, qa_f, 3, 1, 2.0), (2, qa_f, 3, 2, 2.0),
         (3, ah, 1, 0, 1.0), (4, al, 1, 0, 1.0)],
        [(5, 1.0), (6, 1.0)], "a")
    # rhs rows: [bx, by, bz, -256, -1, -256*bh, -bl]
    tb = stage_rows(
        [(0, qb_f, 3, 0, 1.0), (1, qb_f, 3, 1, 1.0), (2, qb_f, 3, 2, 1.0),
         (5, bh, 1, 0, -256.0), (6, bl, 1, 0, -1.0)],
        [(3, -256.0), (4, -1.0)], "b")
    lhsT_all = row_pool.tile([7, NA], bf16)
    rhs_all = row_pool.tile([7, NB], bf16)
    def asm_copy(dst, srcv, i):
        m = i % 3
        if m == 0:
            nc.vector.tensor_copy(dst, srcv)
        elif m == 1:
            nc.scalar.activation(dst, srcv, Act.Copy)
        else:
            nc.gpsimd.tensor_copy(dst, srcv)
    for t in range(NT):
        asm_copy(lhsT_all[:, t * P:(t + 1) * P],
                 ta[t // 4][(t % 4) * 32:(t % 4) * 32 + 7, :], t)
        asm_copy(rhs_all[:, t * P:(t + 1) * P],
                 tb[t // 4][(t % 4) * 32:(t % 4) * 32 + 7, :], t + 1)

    # qb_blocks for exact re-rank: (bx, by, bz, b2) f32 per b point
    qbv = qb_dram.rearrange("g (r d) -> (g r) d", d=4).rearrange(
        "(t p) d -> p t d", p=P)
    nc.sync.dma_start(qbv[:, :, 0:3], qb_f[:].rearrange("p (t d) -> p t d", d=3))
    nc.sync.dma_start(qbv[:, :, 3], b2[:])

    # iotas
    iota16_i = row_pool.tile([P, NC], i32)      # j within group, per candidate
    nc.gpsimd.iota(iota16_i[:], pattern=[[0, NSEL], [1, G]], base=0,
                   channel_multiplier=0)
    iota16 = row_pool.tile([P, NC], f32)
    nc.vector.tensor_copy(iota16[:], iota16_i[:])
    half_c = row_pool.tile([P, 1], f32)
    nc.vector.memset(half_c[:], R)

    # gm-column -> logical-group-id table (gm layout: [DVE h0 | DVE h1 |
    # tree h0 | tree h1] where DVE covers the first ND groups of each half)
    gt_i = row_pool.tile([P, NG], i32)
    nc.gpsimd.iota(gt_i[:, 0:ND], pattern=[[1, ND]], base=0, channel_multiplier=0)
    nc.gpsimd.iota(gt_i[:, ND:2 * ND], pattern=[[1, ND]], base=HG,
                   channel_multiplier=0)
    nc.gpsimd.iota(gt_i[:, 2 * ND:2 * ND + CT], pattern=[[1, CT]], base=ND,
                   channel_multiplier=0)
    nc.gpsimd.iota(gt_i[:, 2 * ND + CT:NG], pattern=[[1, CT]], base=HG + ND,
                   channel_multiplier=0)
    gtable = row_pool.tile([P, NG], f32)
    nc.vector.tensor_copy(gtable[:], gt_i[:])

    qb_blocks = qb_dram.ap()                                   # [256, 64] f32

    # ---------------- Phase 1: chunked, software-pipelined ----------------
    # Issue order per step c: A(c) + qb-gather(c) | B1(c-1) + feat-gather(c-1)
    # | B2(c-2), so each gather's latency hides behind the next chunk's scan.
    ch = ctx.enter_context(tc.tile_pool(name="chunk", bufs=3))
    chb = ctx.enter_context(tc.tile_pool(name="chunkb", bufs=2))
    qdram = [nc.dram_tensor(f"qidx_{c}", [P, SQ], i16) for c in range(NCH)]
    fdram = [nc.dram_tensor(f"fidx_{c}", [P, SF], i16) for c in range(NCH)]

    def wrapped_idx_load(dram_t, idxw):
        # one shuffled read into partitions 0..15, then 7 contiguous replicas
        src = dram_t.ap().rearrange("(r q) s -> q s r", q=16)
        with nc.allow_non_contiguous_dma(reason="tiny idx shuffle"):
            if IDX_DOUBLE == "1":
                nc.sync.dma_start(
                    idxw[0:16, :].rearrange("q (s r) -> q s r", r=8), src)
                for w in (16, 32, 64):
                    nc.sync.dma_start(idxw[w:2 * w, :], idxw[0:w, :])
            elif IDX_DOUBLE == "2":
                nc.sync.dma_start(
                    idxw[0:16, :].rearrange("q (s r) -> q s r", r=8), src)
                for rep in range(1, 8):
                    nc.sync.dma_start(idxw[16 * rep:16 * (rep + 1), :],
                                      idxw[0:16, :])
            elif IDX_DOUBLE == "3":
                nc.sync.dma_start(
                    idxw[0:16, :].rearrange("q (s r) -> q s r", r=8), src)
                for rep in range(1, 8):
                    q = nc.sync if rep % 2 else nc.scalar
                    q.dma_start(idxw[16 * rep:16 * (rep + 1), :],
                                idxw[0:16, :])
            else:
                for rep in range(8):
                    nc.sync.dma_start(
                        idxw[16 * rep:16 * (rep + 1), :].rearrange(
                            "q (s r) -> q s r", r=8), src)

    st = {}

    def stage_a(c):
        g8c = ch.tile([P, TB, NSEL], i16, name=f"g8c_{c}", tag="g8c")
        for tt in range(TB):
            phase_a(nc, tc, sp, psum_pool, lhsT_all, rhs_all, gtable, g8c,
                    c * TB + tt, tt)
        nc.sync.dma_start(qdram[c].ap(), g8c[:].rearrange("p t s -> p (t s)"))
        qidxw = ch.tile([P, SQ * 8], i16, name=f"qiw_{c}", tag="qiw")
        wrapped_idx_load(qdram[c], qidxw)
        st[c] = {"g8c": g8c, "qidxw": qidxw}

    def kick_qb(c):
        s = st[c]
        qcc = ch.tile([P, SQ, D], f32, name=f"qcc_{c}", tag="qcc")
        if os.environ.get("KNOGATHER", "0") == "1":
            nc.vector.memset(qcc[:], 1.0)
        else:
            NSPL = SQ * P // 1024
            for i in range(NSPL):
                w = SQ // NSPL
                nc.gpsimd.dma_gather(qcc[:, i * w:(i + 1) * w, :], qb_blocks,
                                     s["qidxw"][:, i * w * 8:(i + 1) * w * 8],
                                     1024, 1024, D)
        s["qcc"] = qcc

    def stage_b1(c):
        s = st[c]
        g8c, qcc = s["g8c"], s["qcc"]
        m3c = ch.tile([P, TB, K], i16, name=f"m3c_{c}", tag="m3c")
        w3c = ch.tile([P, TB, K], f32, name=f"w3c_{c}", tag="w3c")
        # chunk-wide exact re-rank over [P, TB, NSEL*G] candidates
        qv = qcc[:].rearrange("p (t s) (g d) -> p t (s g) d", t=TB, d=4)
        qa3v = qa_f[:].rearrange("p (t d) -> p t d", d=3)
        cr = chb.tile([P, TB, NC], f32, name=f"cr_{c}", tag="cr")
        t0 = chb.tile([P, TB, NC], f32, name=f"t0_{c}", tag="t0")
        if POOL_B1:
            # per-tile coordinate products + adds on Pool
            for tt in range(TB):
                t = c * TB + tt
                nc.gpsimd.tensor_scalar(cr[:, tt, :], qv[:, tt, :, 0],
                                        qa3v[:, t, 0:1], None, op0=AluOpType.mult)
                nc.gpsimd.tensor_scalar(t0[:, tt, :], qv[:, tt, :, 1],
                                        qa3v[:, t, 1:2], None, op0=AluOpType.mult)
                nc.gpsimd.tensor_tensor(out=cr[:, tt, :], in0=cr[:, tt, :],
                                        in1=t0[:, tt, :], op=AluOpType.add)
                nc.gpsimd.tensor_scalar(t0[:, tt, :], qv[:, tt, :, 2],
                                        qa3v[:, t, 2:3], None, op0=AluOpType.mult)
                nc.gpsimd.tensor_tensor(out=cr[:, tt, :], in0=cr[:, tt, :],
                                        in1=t0[:, tt, :], op=AluOpType.add)
        else:
            qa3c = qa3v[:, c * TB:(c + 1) * TB, :]
            nc.vector.tensor_tensor(
                out=cr[:], in0=qv[:, :, :, 0],
                in1=qa3c[:, :, 0:1].to_broadcast([P, TB, NC]), op=AluOpType.mult)
            nc.vector.tensor_tensor(
                out=t0[:], in0=qv[:, :, :, 1],
                in1=qa3c[:, :, 1:2].to_broadcast([P, TB, NC]), op=AluOpType.mult)
            nc.vector.tensor_tensor(out=cr[:], in0=cr[:], in1=t0[:],
                                    op=AluOpType.add)
            nc.vector.tensor_tensor(
                out=t0[:], in0=qv[:, :, :, 2],
                in1=qa3c[:, :, 2:3].to_broadcast([P, TB, NC]), op=AluOpType.mult)
            nc.vector.tensor_tensor(out=cr[:], in0=cr[:], in1=t0[:],
                                    op=AluOpType.add)
        d2c = chb.tile([P, TB, NC], f32, name=f"d2_{c}", tag="d2")
        nc.vector.scalar_tensor_tensor(out=d2c[:], in0=cr[:], scalar=-2.0,
                                       in1=qv[:, :, :, 3],
                                       op0=AluOpType.mult, op1=AluOpType.add)
        a2c = a2[:, c * TB:(c + 1) * TB]
        nc.vector.tensor_tensor(out=d2c[:], in0=d2c[:],
                                in1=a2c.rearrange("p (t o) -> p t o", o=1).to_broadcast(
                                    [P, TB, NC]), op=AluOpType.add)
        nc.vector.tensor_scalar(d2c[:], d2c[:], 4094.0, None, op0=AluOpType.min)
        # key24 = -(4096*d2 + 16*g + j)
        g8f = chb.tile([P, TB, NSEL], f32, name=f"g8f_{c}", tag="g8f")
        nc.vector.tensor_copy(g8f[:], g8c[:])
        idxc = chb.tile([P, TB, NSEL, G], f32, name=f"ix_{c}", tag="ix")
        nc.vector.scalar_tensor_tensor(
            out=idxc[:],
            in0=g8f[:].rearrange("p t (s o) -> p t s o", o=1).to_broadcast(
                [P, TB, NSEL, G]),
            scalar=16.0,
            in1=iota16[:].rearrange("p (o s g) -> p o s g", o=1, g=G).to_broadcast(
                [P, TB, NSEL, G]),
            op0=AluOpType.mult, op1=AluOpType.add)
        key24 = chb.tile([P, TB, NC], f32, name=f"k24_{c}", tag="k24")
        nc.vector.scalar_tensor_tensor(
            out=key24[:], in0=d2c[:], scalar=-4096.0,
            in1=idxc[:].rearrange("p t s g -> p t (s g)"),
            op0=AluOpType.mult, op1=AluOpType.subtract)
        kvc = chb.tile([P, TB, 8], f32, name=f"kv_{c}", tag="kv")
        for tt in range(TB):
            nc.vector.max(out=kvc[:, tt, :], in_=key24[:, tt, :])
        # decode: r = -key; m = r & 4095; d2 = r >> 12
        r3c = chb.tile([P, TB, K], i32, name=f"r3_{c}", tag="r3")
        nc.gpsimd.tensor_scalar(r3c[:], kvc[:, :, 0:K], -1.0, None,
                                op0=AluOpType.mult)
        m3i = chb.tile([P, TB, K], i32, name=f"m3i_{c}", tag="m3i")
        nc.vector.tensor_scalar(m3i[:], r3c[:], 4095, None,
                                op0=AluOpType.bitwise_and)
        nc.gpsimd.tensor_copy(m3c[:], m3i[:])
        d23i = chb.tile([P, TB, K], i32, name=f"d23i_{c}", tag="d23i")
        nc.vector.tensor_scalar(d23i[:], r3c[:], 12, None,
                                op0=AluOpType.arith_shift_right)
        d23f = chb.tile([P, TB, K], f32, name=f"d23f_{c}", tag="d23f")
        nc.gpsimd.tensor_copy(d23f[:], d23i[:])
        s3c = chb.tile([P, TB, K], f32, name=f"s3_{c}", tag="s3")
        nc.scalar.activation(s3c[:], d23f[:], Act.Sqrt)
        nc.scalar.activation(w3c[:], s3c[:], Act.Relu, scale=-1.0 / FS,
                             bias=half_c[:])
        nc.sync.dma_start(fdram[c].ap(), m3c[:].rearrange("p t k -> p (t k)"))
        fidxw = ch.tile([P, SF * 8], i16, name=f"fiw_{c}", tag="fiw")
        wrapped_idx_load(fdram[c], fidxw)
        st[c].update(w3c=w3c, fidxw=fidxw)

    def kick_ft(c):
        s = st[c]
        gac = ch.tile([P, SF, D], f32, name=f"gac_{c}", tag="gac")
        if os.environ.get("KNOGATHER", "0") == "1":
            nc.vector.memset(gac[:], 1.0)
        else:
            NSPL = SF * P // 1024
            for i in range(NSPL):
                w = SF // NSPL
                nc.gpsimd.dma_gather(gac[:, i * w:(i + 1) * w, :], b_feats,
                                     s["fidxw"][:, i * w * 8:(i + 1) * w * 8],
                                     1024, 1024, D)
        s["gac"] = gac

    def stage_b2(c):
        s = st[c]
        w3c, gac = s["w3c"], s["gac"]
        gv = gac[:].rearrange("p (t k) d -> p t k d", t=TB)
        wm = chb.tile([P, TB, K, D], f32, name=f"wm_{c}", tag="wm")
        if os.environ.get("KWM", "0") == "1":
            nc.gpsimd.tensor_tensor(
                out=wm[:], in0=gv,
                in1=w3c[:].rearrange("p t (k o) -> p t k o", o=1).to_broadcast(
                    [P, TB, K, D]),
                op=AluOpType.mult)
        else:
            nc.vector.tensor_tensor(
                out=wm[:], in0=gv,
                in1=w3c[:].rearrange("p t (k o) -> p t k o", o=1).to_broadcast(
                    [P, TB, K, D]),
                op=AluOpType.mult)
        acc = chb.tile([P, TB, D], f32, name=f"acc_{c}", tag="acc")
        nc.gpsimd.tensor_tensor(out=acc[:], in0=wm[:, :, 0, :], in1=wm[:, :, 1, :],
                                op=AluOpType.add)
        nc.gpsimd.tensor_tensor(out=acc[:], in0=acc[:], in1=wm[:, :, 2, :],
                                op=AluOpType.add)
        nc.sync.dma_start(
            out[c * TB * P:(c + 1) * TB * P, D:2 * D].rearrange(
                "(t p) d -> p t d", p=P), acc[:])
        del st[c]

    for c in range(NCH + 2):
        if c < NCH:
            stage_a(c)
        if 0 <= c - 1 < NCH:
            stage_b1(c - 1)
        if 0 <= c - 2 < NCH:
            stage_b2(c - 2)
        if c < NCH:
            kick_qb(c)
        if 0 <= c - 1 < NCH:
            kick_ft(c - 1)


def phase_a(nc, tc, sp, psum_pool, lhsT_all, rhs_all, gtable, g8c, t, tt):
    # PSUM key drain: DVE reduces first ND groups of each half, Act converts
    # the rest to bf16, DVE runs the pairwise-max tree over the converted slab.
    gm = sp.tile([P, NG], bf16, name=f"gm_{t}", tag="gm")
    gmf = sp.tile([P, 2, ND], f32, name=f"gmf_{t}", tag="gmf")
    kb = sp.tile([P, 2, CT, G], bf16, name=f"kb_{t}", tag="kb")
    for h in range(2):
        ps = psum_pool.tile([P, HALF], f32, name=f"ps_{t}_{h}", tag="ps")
        for j in range(HALF // MM_N):
            nc.tensor.matmul(
                out=ps[:, j * MM_N:(j + 1) * MM_N],
                lhsT=lhsT_all[:, t * P:(t + 1) * P],
                rhs=rhs_all[:, h * HALF + j * MM_N: h * HALF + (j + 1) * MM_N],
                start=True, stop=True,
            )
        psv = ps[:].rearrange("p (g w) -> p g w", w=G)
        nc.vector.tensor_reduce(out=gmf[:, h, :], in_=psv[:, 0:ND, :],
                                axis=X, op=AluOpType.max)
        nc.scalar.activation(kb[:, h, 0:CA, :],
                             psv[:, ND:ND + CA, :].rearrange("p g w -> p (g w)"),
                             Act.Copy)
    # DVE-part group maxima -> gm[:, 0:2*ND]
    nc.vector.tensor_copy(gm[:, 0:2 * ND], gmf[:].rearrange("p h g -> p (h g)"))
    # bf16 pairwise-max tree -> gm[:, 2*ND:]
    if TREE_HALF:
        for h in range(2):
            kbv = kb[:, h]
            r1 = sp.tile([P, CT, 8], bf16, name=f"r1_{t}_{h}", tag=f"r1{h}")
            nc.vector.tensor_tensor(out=r1[:], in0=kbv[:, :, 0:8],
                                    in1=kbv[:, :, 8:16], op=AluOpType.max)
            r2 = sp.tile([P, CT, 4], bf16, name=f"r2_{t}_{h}", tag=f"r2{h}")
            nc.vector.tensor_tensor(out=r2[:], in0=r1[:, :, 0:4], in1=r1[:, :, 4:8],
                                    op=AluOpType.max)
            r3 = sp.tile([P, CT, 2], bf16, name=f"r3_{t}_{h}", tag=f"r3{h}")
            nc.vector.tensor_tensor(out=r3[:], in0=r2[:, :, 0:2], in1=r2[:, :, 2:4],
                                    op=AluOpType.max)
            nc.vector.tensor_tensor(
                out=gm[:, 2 * ND + h * CT:2 * ND + (h + 1) * CT].rearrange(
                    "p (g w) -> p g w", w=1),
                in0=r3[:, :, 0:1], in1=r3[:, :, 1:2], op=AluOpType.max)
    else:
        kbv = kb[:].rearrange("p h g w -> p (h g) w")
        r1 = sp.tile([P, 2 * CT, 8], bf16, name=f"r1_{t}", tag="r1")
        nc.vector.tensor_tensor(out=r1[:], in0=kbv[:, :, 0:8], in1=kbv[:, :, 8:16],
                                op=AluOpType.max)
        r2 = sp.tile([P, 2 * CT, 4], bf16, name=f"r2_{t}", tag="r2")
        nc.vector.tensor_tensor(out=r2[:], in0=r1[:, :, 0:4], in1=r1[:, :, 4:8],
                                op=AluOpType.max)
        r3 = sp.tile([P, 2 * CT, 2], bf16, name=f"r3_{t}", tag="r3")
        nc.vector.tensor_tensor(out=r3[:], in0=r2[:, :, 0:2], in1=r2[:, :, 2:4],
                                op=AluOpType.max)
        nc.vector.tensor_tensor(
            out=gm[:, 2 * ND:NG].rearrange("p (g w) -> p g w", w=1),
            in0=r3[:, :, 0:1], in1=r3[:, :, 1:2], op=AluOpType.max)

    # embed logical group id: gm_g = 256*gm - gtable (f32, 24-bit exact)
    gm_g = sp.tile([P, NG], f32, name=f"gmg_{t}", tag="gmg")
    if os.environ.get("KEMB", "0") == "1":
        nc.gpsimd.tensor_scalar(gm_g[:], gm[:], 256.0, None, op0=AluOpType.mult)
        nc.gpsimd.tensor_tensor(out=gm_g[:], in0=gm_g[:], in1=gtable[:],
                                op=AluOpType.subtract)
    else:
        nc.vector.scalar_tensor_tensor(out=gm_g[:], in0=gm[:], scalar=256.0,
                                       in1=gtable[:], op0=AluOpType.mult,
                                       op1=AluOpType.subtract)
    gv8 = sp.tile([P, 8], f32, name=f"gv8_{t}", tag="gv8")
    nc.vector.max(out=gv8[:], in_=gm_g[:])
    # decode group ids of top NSEL: g = (-v) & 255 -> i16 chunk slice
    rg = sp.tile([P, NSEL], i32, name=f"rg_{t}", tag="rg")
    nc.vector.tensor_scalar(rg[:], gv8[:, 0:NSEL], -1.0, None, op0=AluOpType.mult)
    g8i = sp.tile([P, NSEL], i32, name=f"g8i_{t}", tag="g8i")
    nc.vector.tensor_scalar(g8i[:], rg[:], 255, None, op0=AluOpType.bitwise_and)
    nc.vector.tensor_copy(g8c[:, tt, :], g8i[:])


B = 8
_PROGRAM_CACHE = {}


def build_program():
    if "nc" in _PROGRAM_CACHE:
        return _PROGRAM_CACHE["nc"]
    nc = bacc.Bacc("TRN2", target_bir_lowering=False, debug=False)
    a_feats = nc.dram_tensor("a_feats", [NA, D], f32, kind="ExternalInput").ap()
    b_feats = nc.dram_tensor("b_feats", [NB, D], f32, kind="ExternalInput").ap()
    a_coords = nc.dram_tensor("a_coords", [NA, 3], i32, kind="ExternalInput").ap()
    b_coords = nc.dram_tensor("b_coords", [NB, 3], i32, kind="ExternalInput").ap()
    out = nc.dram_tensor("out", [NA, 2 * D], f32, kind="ExternalOutput").ap()
    with tile.TileContext(nc) as tc:
        knn_kernel(tc, [out], [a_feats, b_feats, a_coords, b_coords])
    nc.compile()
    _PROGRAM_CACHE["nc"] = nc
    return nc


def kernel(a_feats, b_feats, a_coords, b_coords, _trace=False):
    nc = build_program()
    in_maps = [
        {
            "a_feats": np.ascontiguousarray(a_feats[b], dtype=np.float32),
            "b_feats": np.ascontiguousarray(b_feats[b], dtype=np.float32),
            "a_coords": np.ascontiguousarray(a_coords[b], dtype=np.int32),
            "b_coords": np.ascontiguousarray(b_coords[b], dtype=np.int32),
        }
        for b in range(B)
    ]
    res = run_bass_kernel_spmd(nc, in_maps, list(range(B)), trace=_trace)
    out = np.stack([np.asarray(res.results[b]["out"]) for b in range(B)], axis=0)
    if _trace:
        return out.astype(np.float32), res
    return out.astype(np.float32)


# revision 44
# speedup vs baseline: 1.0180x; 1.0180x over previous
"""Trainium2 Bass kernel for retrieval_knn (nn_CLI_63702954934481).

Data-parallel over batch B=8: one batch per NeuronCore. Per core:
7-row bf16 PE matmul computes key = -d2 exactly in PSUM f32 (a^2 and b^2
split into bf16-exact byte halves). Selection: pairwise tensor_tensor max
tree (round 1 f32 PSUM -> bf16 SBUF, later rounds bf16 at 2x DVE rate),
hw top-8 max/max_index over group maxima, exact re-rank of the top NSEL
groups' candidates with full (d2, index) tie-breaking, indirect-DMA
gathers, weighted sum, concat. Self-contained.
"""
import sys

if "/opt/trn_rl_repo" not in sys.path:
    sys.path.insert(0, "/opt/trn_rl_repo")

import numpy as np
from concourse import bacc
from concourse.bass_utils import run_bass_kernel_spmd

from contextlib import ExitStack

import concourse.bass as bass
import concourse.mybir as mybir
import concourse.tile as tile
from concourse._compat import with_exitstack
from concourse.alu_op_type import AluOpType

P = 128
NA = 4096
NB = 4096
D = 64
K = 3
NT = NA // P      # 32 a-tiles
G = 16            # group size along b
NG = NB // G      # 256 groups
import os
NSEL = 4          # groups re-ranked per a-row
NC = NSEL * G     # 64 candidates
FS = 128.0
R = 0.5

MM_N = 512        # matmul chunk (one PSUM bank)
import os
NH = int(os.environ.get("KNH", "4"))   # PSUM pieces per tile
HALF = NB // NH   # b-columns per PSUM tile
HG = HALF // G    # groups per piece
ND = int(os.environ.get("KND", "10"))  # DVE-reduce groups per piece
IDX_DOUBLE = os.environ.get("KIDXD", "2")
POOL_B1 = os.environ.get("KPOOLB1", "0") == "1"
TREE_HALF = os.environ.get("KTREEH", "0") == "1"
CA = HG - ND      # groups per half converted to bf16 by Act (Pool can't
CT = CA           # read PSUM; the drain is DVE+Act only)

TB = int(os.environ.get("KTB", "8"))  # tiles per gather chunk
NCH = NT // TB    # chunks
SQ = TB * NSEL    # qb-window slots per chunk
SF = TB * K       # feature slots per chunk

f32 = mybir.dt.float32
i32 = mybir.dt.int32
i16 = mybir.dt.int16
u32 = mybir.dt.uint32
bf16 = mybir.dt.bfloat16
Act = mybir.ActivationFunctionType
X = mybir.AxisListType.X


@with_exitstack
def knn_kernel(ctx: ExitStack, tc: tile.TileContext, outs, ins):
    nc = tc.nc
    a_feats, b_feats, a_coords, b_coords = ins
    out = outs[0]

    prep_pool = ctx.enter_context(tc.tile_pool(name="prep", bufs=1))
    row_pool = ctx.enter_context(tc.tile_pool(name="rows", bufs=1))
    psum_pool = ctx.enter_context(tc.tile_pool(name="ps", bufs=NH, space="PSUM"))
    sp = ctx.enter_context(tc.tile_pool(name="small", bufs=int(os.environ.get("KSPB", "4"))))

    # ---------------- Phase 0: prep ----------------
    # lhsT rows (per a): [2ax, 2ay, 2az, ah, al, 1, 1]
    # rhs rows (per b):  [bx, by, bz, -256, -1, -256*bh, -bl]
    # PSUM key = 2a.b - a^2 - b^2 = -d2 (exact f32 integer)
    qb_dram = nc.dram_tensor("qb_scratch", [NG, G * 4], f32)

    def load_q(coords, tag):
        c = prep_pool.tile([P, NT * 3], i32, name=f"c_{tag}")
        nc.sync.dma_start(c[:], coords.rearrange("(t p) d -> p t d", p=P))
        q_i = prep_pool.tile([P, NT * 3], i32, name=f"qi_{tag}")
        nc.vector.tensor_scalar(q_i[:], c[:], 4, None, op0=AluOpType.arith_shift_right)
        q_f = prep_pool.tile([P, NT * 3], f32, name=f"qf_{tag}")
        nc.vector.tensor_copy(q_f[:], q_i[:])
        return q_f

    def sq_sum(q_f, tag):
        sq = prep_pool.tile([P, NT * 3], f32, name=f"sq_{tag}")
        nc.vector.tensor_tensor(out=sq[:], in0=q_f[:], in1=q_f[:], op=AluOpType.mult)
        s2 = prep_pool.tile([P, NT], f32, name=f"s2_{tag}")
        nc.vector.tensor_reduce(out=s2[:], in_=sq[:].rearrange("p (t d) -> p t d", d=3),
                                axis=X, op=AluOpType.add)
        return s2

    def split_bytes(s2, tag):
        # s2 integer-valued f32 in [0, 49152): hi = s2>>8, lo = s2&255 (f32)
        s2_i = prep_pool.tile([P, NT], i32, name=f"s2i_{tag}")
        nc.vector.tensor_copy(s2_i[:], s2[:])
        hi_i = prep_pool.tile([P, NT], i32, name=f"hi_{tag}")
        nc.vector.tensor_scalar(hi_i[:], s2_i[:], 8, None, op0=AluOpType.arith_shift_right)
        lo_i = prep_pool.tile([P, NT], i32, name=f"lo_{tag}")
        nc.vector.tensor_scalar(lo_i[:], s2_i[:], 255, None, op0=AluOpType.bitwise_and)
        hi_f = prep_pool.tile([P, NT], f32, name=f"hif_{tag}")
        nc.vector.tensor_copy(hi_f[:], hi_i[:])
        lo_f = prep_pool.tile([P, NT], f32, name=f"lof_{tag}")
        nc.vector.tensor_copy(lo_f[:], lo_i[:])
        return hi_f, lo_f

    def stage_rows(vals_scales, const_rows, tag):
        # build [P, NT, 32] bf16 staging: col r holds row-r value per point;
        # stride 32 so transposed slices land on legal partition bases
        stg = prep_pool.tile([P, NT, 32], bf16, name=f"stg_{tag}")
        sv = stg[:]
        nc.gpsimd.memset(stg[:].rearrange("p t r -> p (t r)"), 0.0)
        for r, src_f, kk, dsel, scale in vals_scales:
            if kk == 1:
                view = src_f[:]
            else:
                view = src_f[:].rearrange("p (t d) -> p t d", d=kk)[:, :, dsel]
            nc.vector.tensor_scalar(sv[:, :, r], view, scale, None,
                                    op0=AluOpType.mult)
        for r, val in const_rows:
            nc.vector.memset(sv[:, :, r], val)
        # 8 XBAR transposes -> [(t r), p], 4 point-tiles per transpose
        trs = []
        for i in range(NT * 32 // P):
            tr = row_pool.tile([P, P], bf16, name=f"tr{i}_{tag}")
            nc.sync.dma_start_transpose(
                tr[:], stg[:].rearrange("p t r -> p (t r)")[:, i * P:(i + 1) * P])
            trs.append(tr)
        return trs

    nc.sync.dma_start(out[:, 0:D], a_feats[:])
    qa_f = load_q(a_coords, "a")
    qb_f = load_q(b_coords, "b")
    a2 = sq_sum(qa_f, "a")
    b2 = sq_sum(qb_f, "b")
    ah, al = split_bytes(a2, "a")
    bh, bl = split_bytes(b2, "b")

    # lhsT rows: [2ax, 2ay, 2az, ah, al, 1, 1]
    ta = stage_rows(
        [(0, qa_f, 3, 0, 2.0), (1, qa_f, 3, 1, 2.0), (2, qa_f, 3, 2, 2.0),
         (3, ah, 1, 0, 1.0), (4, al, 1, 0, 1.0)],
        [(5, 1.0), (6, 1.0)], "a")
    # rhs rows: [bx, by, bz, -256, -1, -256*bh, -bl]
    tb = stage_rows(
        [(0, qb_f, 3, 0, 1.0), (1, qb_f, 3, 1, 1.0), (2, qb_f, 3, 2, 1.0),
         (5, bh, 1, 0, -256.0), (6, bl, 1, 0, -1.0)],
        [(3, -256.0), (4, -1.0)], "b")
    lhsT_all = row_pool.tile([7, NA], bf16)
    rhs_all = row_pool.tile([7, NB], bf16)
    def asm_copy(dst, srcv, i):
        m = i % 3
        if m == 0:
            nc.vector.tensor_copy(dst, srcv)
        elif m == 1:
            nc.scalar.activation(dst, srcv, Act.Copy)
        else:
            nc.gpsimd.tensor_copy(dst, srcv)
    for t in range(NT):
        asm_copy(lhsT_all[:, t * P:(t + 1) * P],
                 ta[t // 4][(t % 4) * 32:(t % 4) * 32 + 7, :], t)
        asm_copy(rhs_all[:, t * P:(t + 1) * P],
                 tb[t // 4][(t % 4) * 32:(t % 4) * 32 + 7, :], t + 1)

    # qb_blocks for exact re-rank: (bx, by, bz, b2) f32 per b point
    qbv = qb_dram.rearrange("g (r d) -> (g r) d", d=4).rearrange(
        "(t p) d -> p t d", p=P)
    nc.sync.dma_start(qbv[:, :, 0:3], qb_f[:].rearrange("p (t d) -> p t d", d=3))
    nc.sync.dma_start(qbv[:, :, 3], b2[:])

    # iotas
    iota16_i = row_pool.tile([P, NC], i32)      # j within group, per candidate
    nc.gpsimd.iota(iota16_i[:], pattern=[[0, NSEL], [1, G]], base=0,
                   channel_multiplier=0)
    iota16 = row_pool.tile([P, NC], f32)
    nc.vector.tensor_copy(iota16[:], iota16_i[:])
    half_c = row_pool.tile([P, 1], f32)
    nc.vector.memset(half_c[:], R)

    # gm-column -> logical-group-id table (gm layout: [DVE h0 | DVE h1 |
    # tree h0 | tree h1] where DVE covers the first ND groups of each half)
    gt_i = row_pool.tile([P, NG], i32)
    for h in range(NH):
        if ND:
            nc.gpsimd.iota(gt_i[:, h * ND:(h + 1) * ND], pattern=[[1, ND]],
                           base=h * HG, channel_multiplier=0)
        nc.gpsimd.iota(gt_i[:, NH * ND + h * CT:NH * ND + (h + 1) * CT],
                       pattern=[[1, CT]], base=h * HG + ND, channel_multiplier=0)
    gtable = row_pool.tile([P, NG], f32)
    nc.vector.tensor_copy(gtable[:], gt_i[:])

    qb_blocks = qb_dram.ap()                                   # [256, 64] f32

    # ---------------- Phase 1: chunked, software-pipelined ----------------
    # Issue order per step c: A(c) + qb-gather(c) | B1(c-1) + feat-gather(c-1)
    # | B2(c-2), so each gather's latency hides behind the next chunk's scan.
    ch = ctx.enter_context(tc.tile_pool(name="chunk", bufs=3))
    chb = ctx.enter_context(tc.tile_pool(name="chunkb", bufs=2))
    qdram = [nc.dram_tensor(f"qidx_{c}", [P, SQ], i16) for c in range(NCH)]
    fdram = [nc.dram_tensor(f"fidx_{c}", [P, SF], i16) for c in range(NCH)]

    def wrapped_idx_load(dram_t, idxw):
        # one shuffled read into partitions 0..15, then 7 contiguous replicas
        src = dram_t.ap().rearrange("(r q) s -> q s r", q=16)
        with nc.allow_non_contiguous_dma(reason="tiny idx shuffle"):
            if IDX_DOUBLE == "1":
                nc.sync.dma_start(
                    idxw[0:16, :].rearrange("q (s r) -> q s r", r=8), src)
                for w in (16, 32, 64):
                    nc.sync.dma_start(idxw[w:2 * w, :], idxw[0:w, :])
            elif IDX_DOUBLE == "2":
                nc.sync.dma_start(
                    idxw[0:16, :].rearrange("q (s r) -> q s r", r=8), src)
                for rep in range(1, 8):
                    nc.sync.dma_start(idxw[16 * rep:16 * (rep + 1), :],
                                      idxw[0:16, :])
            elif IDX_DOUBLE == "3":
                nc.sync.dma_start(
                    idxw[0:16, :].rearrange("q (s r) -> q s r", r=8), src)
                for rep in range(1, 8):
                    q = nc.sync if rep % 2 else nc.scalar
                    q.dma_start(idxw[16 * rep:16 * (rep + 1), :],
                                idxw[0:16, :])
            else:
                for rep in range(8):
                    nc.sync.dma_start(
                        idxw[16 * rep:16 * (rep + 1), :].rearrange(
                            "q (s r) -> q s r", r=8), src)

    st = {}

    def stage_a(c):
        g8c = ch.tile([P, TB, NSEL], i16, name=f"g8c_{c}", tag="g8c")
        for tt in range(TB):
            phase_a(nc, tc, sp, psum_pool, lhsT_all, rhs_all, gtable, g8c,
                    c * TB + tt, tt)
        nc.sync.dma_start(qdram[c].ap(), g8c[:].rearrange("p t s -> p (t s)"))
        qidxw = ch.tile([P, SQ * 8], i16, name=f"qiw_{c}", tag="qiw")
        wrapped_idx_load(qdram[c], qidxw)
        st[c] = {"g8c": g8c, "qidxw": qidxw}

    def kick_qb(c):
        s = st[c]
        qcc = ch.tile([P, SQ, D], f32, name=f"qcc_{c}", tag="qcc")
        if os.environ.get("KNOGATHER", "0") == "1":
            nc.vector.memset(qcc[:], 1.0)
        else:
            NSPL = SQ * P // 1024
            for i in range(NSPL):
                w = SQ // NSPL
                nc.gpsimd.dma_gather(qcc[:, i * w:(i + 1) * w, :], qb_blocks,
                                     s["qidxw"][:, i * w * 8:(i + 1) * w * 8],
                                     1024, 1024, D)
        s["qcc"] = qcc

    def stage_b1(c):
        s = st[c]
        g8c, qcc = s["g8c"], s["qcc"]
        m3c = ch.tile([P, TB, K], i16, name=f"m3c_{c}", tag="m3c")
        w3c = ch.tile([P, TB, K], f32, name=f"w3c_{c}", tag="w3c")
        # chunk-wide exact re-rank over [P, TB, NSEL*G] candidates
        qv = qcc[:].rearrange("p (t s) (g d) -> p t (s g) d", t=TB, d=4)
        qa3v = qa_f[:].rearrange("p (t d) -> p t d", d=3)
        cr = chb.tile([P, TB, NC], f32, name=f"cr_{c}", tag="cr")
        t0 = chb.tile([P, TB, NC], f32, name=f"t0_{c}", tag="t0")
        if POOL_B1:
            # per-tile coordinate products + adds on Pool
            for tt in range(TB):
                t = c * TB + tt
                nc.gpsimd.tensor_scalar(cr[:, tt, :], qv[:, tt, :, 0],
                                        qa3v[:, t, 0:1], None, op0=AluOpType.mult)
                nc.gpsimd.tensor_scalar(t0[:, tt, :], qv[:, tt, :, 1],
                                        qa3v[:, t, 1:2], None, op0=AluOpType.mult)
                nc.gpsimd.tensor_tensor(out=cr[:, tt, :], in0=cr[:, tt, :],
                                        in1=t0[:, tt, :], op=AluOpType.add)
                nc.gpsimd.tensor_scalar(t0[:, tt, :], qv[:, tt, :, 2],
                                        qa3v[:, t, 2:3], None, op0=AluOpType.mult)
                nc.gpsimd.tensor_tensor(out=cr[:, tt, :], in0=cr[:, tt, :],
                                        in1=t0[:, tt, :], op=AluOpType.add)
        else:
            qa3c = qa3v[:, c * TB:(c + 1) * TB, :]
            nc.vector.tensor_tensor(
                out=cr[:], in0=qv[:, :, :, 0],
                in1=qa3c[:, :, 0:1].to_broadcast([P, TB, NC]), op=AluOpType.mult)
            nc.vector.tensor_tensor(
                out=t0[:], in0=qv[:, :, :, 1],
                in1=qa3c[:, :, 1:2].to_broadcast([P, TB, NC]), op=AluOpType.mult)
            nc.vector.tensor_tensor(out=cr[:], in0=cr[:], in1=t0[:],
                                    op=AluOpType.add)
            nc.vector.tensor_tensor(
                out=t0[:], in0=qv[:, :, :, 2],
                in1=qa3c[:, :, 2:3].to_broadcast([P, TB, NC]), op=AluOpType.mult)
            nc.vector.tensor_tensor(out=cr[:], in0=cr[:], in1=t0[:],
                                    op=AluOpType.add)
        d2c = chb.tile([P, TB, NC], f32, name=f"d2_{c}", tag="d2")
        nc.vector.scalar_tensor_tensor(out=d2c[:], in0=cr[:], scalar=-2.0,
                                       in1=qv[:, :, :, 3],
                                       op0=AluOpType.mult, op1=AluOpType.add)
        a2c = a2[:, c * TB:(c + 1) * TB]
        nc.vector.tensor_tensor(out=d2c[:], in0=d2c[:],
                                in1=a2c.rearrange("p (t o) -> p t o", o=1).to_broadcast(
                                    [P, TB, NC]), op=AluOpType.add)
        nc.vector.tensor_scalar(d2c[:], d2c[:], 4094.0, None, op0=AluOpType.min)
        # key24 = -(4096*d2 + 16*g + j)
        g8f = chb.tile([P, TB, NSEL], f32, name=f"g8f_{c}", tag="g8f")
        nc.vector.tensor_copy(g8f[:], g8c[:])
        idxc = chb.tile([P, TB, NSEL, G], f32, name=f"ix_{c}", tag="ix")
        nc.vector.scalar_tensor_tensor(
            out=idxc[:],
            in0=g8f[:].rearrange("p t (s o) -> p t s o", o=1).to_broadcast(
                [P, TB, NSEL, G]),
            scalar=16.0,
            in1=iota16[:].rearrange("p (o s g) -> p o s g", o=1, g=G).to_broadcast(
                [P, TB, NSEL, G]),
            op0=AluOpType.mult, op1=AluOpType.add)
        key24 = chb.tile([P, TB, NC], f32, name=f"k24_{c}", tag="k24")
        nc.vector.scalar_tensor_tensor(
            out=key24[:], in0=d2c[:], scalar=-4096.0,
            in1=idxc[:].rearrange("p t s g -> p t (s g)"),
            op0=AluOpType.mult, op1=AluOpType.subtract)
        kvc = chb.tile([P, TB, 8], f32, name=f"kv_{c}", tag="kv")
        for tt in range(TB):
            nc.vector.max(out=kvc[:, tt, :], in_=key24[:, tt, :])
        # decode: r = -key; m = r & 4095; d2 = r >> 12
        r3c = chb.tile([P, TB, K], i32, name=f"r3_{c}", tag="r3")
        nc.gpsimd.tensor_scalar(r3c[:], kvc[:, :, 0:K], -1.0, None,
                                op0=AluOpType.mult)
        m3i = chb.tile([P, TB, K], i32, name=f"m3i_{c}", tag="m3i")
        nc.vector.tensor_scalar(m3i[:], r3c[:], 4095, None,
                                op0=AluOpType.bitwise_and)
        nc.gpsimd.tensor_copy(m3c[:], m3i[:])
        d23i = chb.tile([P, TB, K], i32, name=f"d23i_{c}", tag="d23i")
        nc.vector.tensor_scalar(d23i[:], r3c[:], 12, None,
                                op0=AluOpType.arith_shift_right)
        d23f = chb.tile([P, TB, K], f32, name=f"d23f_{c}", tag="d23f")
        nc.gpsimd.tensor_copy(d23f[:], d23i[:])
        s3c = chb.tile([P, TB, K], f32, name=f"s3_{c}", tag="s3")
        nc.scalar.activation(s3c[:], d23f[:], Act.Sqrt)
        nc.scalar.activation(w3c[:], s3c[:], Act.Relu, scale=-1.0 / FS,
                             bias=half_c[:])
        nc.sync.dma_start(fdram[c].ap(), m3c[:].rearrange("p t k -> p (t k)"))
        fidxw = ch.tile([P, SF * 8], i16, name=f"fiw_{c}", tag="fiw")
        wrapped_idx_load(fdram[c], fidxw)
        st[c].update(w3c=w3c, fidxw=fidxw)

    def kick_ft(c):
        s = st[c]
        gac = ch.tile([P, SF, D], f32, name=f"gac_{c}", tag="gac")
        if os.environ.get("KNOGATHER", "0") == "1":
            nc.vector.memset(gac[:], 1.0)
        else:
            NSPL = SF * P // 1024
            for i in range(NSPL):
                w = SF // NSPL
                nc.gpsimd.dma_gather(gac[:, i * w:(i + 1) * w, :], b_feats,
                                     s["fidxw"][:, i * w * 8:(i + 1) * w * 8],
                                     1024, 1024, D)
        s["gac"] = gac

    def stage_b2(c):
        s = st[c]
        w3c, gac = s["w3c"], s["gac"]
        gv = gac[:].rearrange("p (t k) d -> p t k d", t=TB)
        wm = chb.tile([P, TB, K, D], f32, name=f"wm_{c}", tag="wm")
        if os.environ.get("KWM", "0") == "1":
            nc.gpsimd.tensor_tensor(
                out=wm[:], in0=gv,
                in1=w3c[:].rearrange("p t (k o) -> p t k o", o=1).to_broadcast(
                    [P, TB, K, D]),
                op=AluOpType.mult)
        else:
            nc.vector.tensor_tensor(
                out=wm[:], in0=gv,
                in1=w3c[:].rearrange("p t (k o) -> p t k o", o=1).to_broadcast(
                    [P, TB, K, D]),
                op=AluOpType.mult)
        acc = chb.tile([P, TB, D], f32, name=f"acc_{c}", tag="acc")
        nc.gpsimd.tensor_tensor(out=acc[:], in0=wm[:, :, 0, :], in1=wm[:, :, 1, :],
                                op=AluOpType.add)
        nc.gpsimd.tensor_tensor(out=acc[:], in0=acc[:], in1=wm[:, :, 2, :],
                                op=AluOpType.add)
        nc.sync.dma_start(
            out[c * TB * P:(c + 1) * TB * P, D:2 * D].rearrange(
                "(t p) d -> p t d", p=P), acc[:])
        del st[c]

    for c in range(NCH + 2):
        if c < NCH:
            stage_a(c)
        if 0 <= c - 1 < NCH:
            stage_b1(c - 1)
        if 0 <= c - 2 < NCH:
            stage_b2(c - 2)
        if c < NCH:
            kick_qb(c)
        if 0 <= c - 1 < NCH:
            kick_ft(c - 1)


def phase_a(nc, tc, sp, psum_pool, lhsT_all, rhs_all, gtable, g8c, t, tt):
    # PSUM key drain: DVE reduces first ND groups of each half, Act converts
    # the rest to bf16, DVE runs the pairwise-max tree over the converted slab.
    gm = sp.tile([P, NG], bf16, name=f"gm_{t}", tag="gm")
    gmf = sp.tile([P, NH, max(ND, 1)], f32, name=f"gmf_{t}", tag="gmf")
    kb = sp.tile([P, NH, CT, G], bf16, name=f"kb_{t}", tag="kb")
    for h in range(NH):
        ps = psum_pool.tile([P, HALF], f32, name=f"ps_{t}_{h}", tag="ps")
        for j in range(HALF // MM_N):
            nc.tensor.matmul(
                out=ps[:, j * MM_N:(j + 1) * MM_N],
                lhsT=lhsT_all[:, t * P:(t + 1) * P],
                rhs=rhs_all[:, h * HALF + j * MM_N: h * HALF + (j + 1) * MM_N],
                start=True, stop=True,
            )
        psv = ps[:].rearrange("p (g w) -> p g w", w=G)
        if ND:
            nc.vector.tensor_reduce(out=gmf[:, h, :], in_=psv[:, 0:ND, :],
                                    axis=X, op=AluOpType.max)
        nc.scalar.activation(kb[:, h, 0:CA, :],
                             psv[:, ND:ND + CA, :].rearrange("p g w -> p (g w)"),
                             Act.Copy)
    # DVE-part group maxima -> gm[:, 0:NH*ND]
    if ND:
        nc.vector.tensor_copy(gm[:, 0:NH * ND],
                              gmf[:].rearrange("p h g -> p (h g)"))
    # bf16 pairwise-max tree -> gm[:, 2*ND:]
    if TREE_HALF:
        for h in range(NH):
            kbv = kb[:, h]
            r1 = sp.tile([P, CT, 8], bf16, name=f"r1_{t}_{h}", tag=f"r1{h}")
            nc.vector.tensor_tensor(out=r1[:], in0=kbv[:, :, 0:8],
                                    in1=kbv[:, :, 8:16], op=AluOpType.max)
            r2 = sp.tile([P, CT, 4], bf16, name=f"r2_{t}_{h}", tag=f"r2{h}")
            nc.vector.tensor_tensor(out=r2[:], in0=r1[:, :, 0:4], in1=r1[:, :, 4:8],
                                    op=AluOpType.max)
            r3 = sp.tile([P, CT, 2], bf16, name=f"r3_{t}_{h}", tag=f"r3{h}")
            nc.vector.tensor_tensor(out=r3[:], in0=r2[:, :, 0:2], in1=r2[:, :, 2:4],
                                    op=AluOpType.max)
            nc.vector.tensor_tensor(
                out=gm[:, NH * ND + h * CT:NH * ND + (h + 1) * CT].rearrange(
                    "p (g w) -> p g w", w=1),
                in0=r3[:, :, 0:1], in1=r3[:, :, 1:2], op=AluOpType.max)
    else:
        kbv = kb[:].rearrange("p h g w -> p (h g) w")
        r1 = sp.tile([P, NH * CT, 8], bf16, name=f"r1_{t}", tag="r1")
        nc.vector.tensor_tensor(out=r1[:], in0=kbv[:, :, 0:8], in1=kbv[:, :, 8:16],
                                op=AluOpType.max)
        r2 = sp.tile([P, NH * CT, 4], bf16, name=f"r2_{t}", tag="r2")
        nc.vector.tensor_tensor(out=r2[:], in0=r1[:, :, 0:4], in1=r1[:, :, 4:8],
                                op=AluOpType.max)
        r3 = sp.tile([P, NH * CT, 2], bf16, name=f"r3_{t}", tag="r3")
        nc.vector.tensor_tensor(out=r3[:], in0=r2[:, :, 0:2], in1=r2[:, :, 2:4],
                                op=AluOpType.max)
        nc.vector.tensor_tensor(
            out=gm[:, NH * ND:NG].rearrange("p (g w) -> p g w", w=1),
            in0=r3[:, :, 0:1], in1=r3[:, :, 1:2], op=AluOpType.max)

    # embed logical group id: gm_g = 256*gm - gtable (f32, 24-bit exact)
    gm_g = sp.tile([P, NG], f32, name=f"gmg_{t}", tag="gmg")
    if os.environ.get("KEMB", "0") == "1":
        nc.gpsimd.tensor_scalar(gm_g[:], gm[:], 256.0, None, op0=AluOpType.mult)
        nc.gpsimd.tensor_tensor(out=gm_g[:], in0=gm_g[:], in1=gtable[:],
                                op=AluOpType.subtract)
    else:
        nc.vector.scalar_tensor_tensor(out=gm_g[:], in0=gm[:], scalar=256.0,
                                       in1=gtable[:], op0=AluOpType.mult,
                                       op1=AluOpType.subtract)
    gv8 = sp.tile([P, 8], f32, name=f"gv8_{t}", tag="gv8")
    nc.vector.max(out=gv8[:], in_=gm_g[:])
    # decode group ids of top NSEL: g = (-v) & 255 -> i16 chunk slice
    rg = sp.tile([P, NSEL], i32, name=f"rg_{t}", tag="rg")
    nc.vector.tensor_scalar(rg[:], gv8[:, 0:NSEL], -1.0, None, op0=AluOpType.mult)
    g8i = sp.tile([P, NSEL], i32, name=f"g8i_{t}", tag="g8i")
    nc.vector.tensor_scalar(g8i[:], rg[:], 255, None, op0=AluOpType.bitwise_and)
    nc.vector.tensor_copy(g8c[:, tt, :], g8i[:])


B = 8
_PROGRAM_CACHE = {}


def build_program():
    if "nc" in _PROGRAM_CACHE:
        return _PROGRAM_CACHE["nc"]
    nc = bacc.Bacc("TRN2", target_bir_lowering=False, debug=False)
    a_feats = nc.dram_tensor("a_feats", [NA, D], f32, kind="ExternalInput").ap()
    b_feats = nc.dram_tensor("b_feats", [NB, D], f32, kind="ExternalInput").ap()
    a_coords = nc.dram_tensor("a_coords", [NA, 3], i32, kind="ExternalInput").ap()
    b_coords = nc.dram_tensor("b_coords", [NB, 3], i32, kind="ExternalInput").ap()
    out = nc.dram_tensor("out", [NA, 2 * D], f32, kind="ExternalOutput").ap()
    with tile.TileContext(nc) as tc:
        knn_kernel(tc, [out], [a_feats, b_feats, a_coords, b_coords])
    nc.compile()
    _PROGRAM_CACHE["nc"] = nc
    return nc


def kernel(a_feats, b_feats, a_coords, b_coords, _trace=False):
    nc = build_program()
    in_maps = [
        {
            "a_feats": np.ascontiguousarray(a_feats[b], dtype=np.float32),
            "b_feats": np.ascontiguousarray(b_feats[b], dtype=np.float32),
            "a_coords": np.ascontiguousarray(a_coords[b], dtype=np.int32),
            "b_coords": np.ascontiguousarray(b_coords[b], dtype=np.int32),
        }
        for b in range(B)
    ]
    res = run_bass_kernel_spmd(nc, in_maps, list(range(B)), trace=_trace)
    out = np.stack([np.asarray(res.results[b]["out"]) for b in range(B)], axis=0)
    if _trace:
        return out.astype(np.float32), res
    return out.astype(np.float32)
